# revision 15
# baseline (speedup 1.0000x reference)
"""ContrastiveCenterLoss Trainium2 Bass kernel — gather-free formulation.

Math (exact identities; see git history for derivation):
    cn_c = fc_c/||fc_c||;  s = sum_c cn_c;  X_b = hidden_b/||hidden_b||
    G_c  = sum_{b: y_b=c} X_b
    loss = 1 + sum_c G_c . (s - C*cn_c) / (B*(C-1))

The host relabels classes as n = 8*pos + w (window w, slot pos) so the
fc load is contiguous per partition, and bins each core's samples so
tile j only holds classes of window j//5. G accumulates on the PE via
fp8 one-hot matmuls (one-hot precomputed host-side — pure index
preprocessing). Pad slots duplicate a real row with a zero one-hot row.

DMA: h in 3 big chunks on sync (big chunks ~350GB/s vs 70GB/s for
small), fc contiguous on scalar, one-hot fp8 on the gpsimd SWDGE queue.
"""

import sys

sys.path.insert(0, "/opt/trn_rl_repo")

from contextlib import ExitStack

import numpy as np

import concourse.bass as bass
import concourse.tile as tile
from concourse import bacc, mybir

B, C, D = 32768, 1000, 128
NCORES = 8
W = 8                     # class windows
CPW = C // W              # 125 classes per window
TPW = 5                   # tiles per window
NT = W * TPW              # 40 tiles of 128 slots per core
BS_PAD = NT * 128         # 5120 padded slots per core
GROUPS = [(0, 10), (10, 20), (20, 30), (30, 40)]
HCHUNKS = [(0, 20), (20, 40)]
F32 = mybir.dt.float32
BF16 = mybir.dt.bfloat16
FP8 = mybir.dt.float8e4

_CACHED_NC = None


def build_nc() -> bass.Bass:
    AF = mybir.ActivationFunctionType
    OP = mybir.AluOpType

    nc = bacc.Bacc()
    hidden = nc.dram_tensor("hidden", [BS_PAD, D], F32, kind="ExternalInput")
    fc = nc.dram_tensor("fc", [C, D], F32, kind="ExternalInput")
    oh_t = nc.dram_tensor("oh", [128, NT, CPW], FP8, kind="ExternalInput")
    out_res = nc.dram_tensor("res", [128, 2], F32, kind="ExternalOutput")

    with tile.TileContext(nc) as tc, ExitStack() as ctx:
        singles = ctx.enter_context(tc.tile_pool(name="singles", bufs=1))
        psum = ctx.enter_context(tc.tile_pool(name="psum", bufs=1, space="PSUM"))

        # ---------------- DMA in ----------------
        # fc FIRST on sync (its arrival gates the centers->T->finals chain);
        # h group-chunks split across both HWDGE queues; one-hot on SWDGE.
        fc_sb = singles.tile([CPW, W, D], F32)
        nc.scalar.dma_start(
            out=fc_sb, in_=fc[:, :].rearrange("(p t) d -> p t d", t=W)
        )

        h_all = singles.tile([128, NT, D], F32)
        h_src = hidden[:, :].rearrange("(p i) d -> p i d", p=128)
        for j0, j1 in HCHUNKS:
            nc.sync.dma_start(out=h_all[:, j0:j1, :], in_=h_src[:, j0:j1, :])

        oh = singles.tile([128, NT, CPW], FP8)
        nc.gpsimd.dma_start(out=oh[:, 0:24, :], in_=oh_t[:, 0:24, :])
        nc.gpsimd.dma_start(out=oh[:, 24:NT, :], in_=oh_t[:, 24:NT, :])

        # Preload both ACT tables during the DMA window.
        warm = singles.tile([128, 1], F32)
        nc.scalar.activation(out=warm, in_=warm, func=AF.Sqrt, scale=0.0, bias=1.0)
        nc.scalar.activation(out=warm, in_=warm, func=AF.Square)

        # ---------------- tiles ----------------
        sq = singles.tile([128, NT, D], BF16)
        q_all = singles.tile([128, NT], BF16)
        rt_all = singles.tile([128, NT], BF16)
        invh = singles.tile([128, NT], BF16)
        x8 = singles.tile([128, NT, D], FP8)
        g_ps = psum.tile([CPW, W, D], F32)
        s_ps = psum.tile([128, D], F32)
        res_sb = singles.tile([128, 2], F32)
        nc.vector.memset(res_sb, 0.0)
        scratch = singles.tile([CPW, W, D], F32)
        T_sb = singles.tile([CPW, W, D], F32)

        sq_c = singles.tile([CPW, W, D], BF16)
        q_c = singles.tile([CPW, W], F32)
        rt_c = singles.tile([CPW, W], F32)
        inv_c = singles.tile([CPW, W], F32)
        cn_bf = singles.tile([CPW, W, D], BF16)
        ones_bf = singles.tile([CPW, 128], BF16)
        nc.vector.memset(ones_bf, 1.0)

        def emit_group(gi):
            j0, j1 = GROUPS[gi]
            n = j1 - j0
            nc.scalar.activation(
                out=sq[:, j0:j1, :], in_=h_all[:, j0:j1, :], func=AF.Square
            )
            with nc.allow_low_precision("q in bf16: 0.4% on norms is fine"):
                nc.vector.tensor_reduce(
                    out=q_all[:, j0:j1],
                    in_=sq[:, j0:j1, :],
                    axis=mybir.AxisListType.X,
                    op=OP.add,
                )
            nc.scalar.activation(
                out=rt_all[:, j0:j1], in_=q_all[:, j0:j1], func=AF.Sqrt
            )
            with nc.allow_low_precision("invh in bf16: fine for cosine tol"):
                nc.vector.reciprocal(out=invh[:, j0:j1], in_=rt_all[:, j0:j1])
            # X = h/||h|| in fp8 (PE operand); g0/g1 on gpsimd, g2/g3 DVE
            eng = nc.gpsimd if gi < 2 else nc.vector
            eng.tensor_tensor(
                out=x8[:, j0:j1, :],
                in0=h_all[:, j0:j1, :],
                in1=invh[:, j0:j1].broadcast_to([128, n, D]),
                op=OP.mult,
            )
            for j in range(j0, j1):
                w = j // TPW
                nc.tensor.matmul(
                    out=g_ps[:, w, :],
                    lhsT=oh[:, j, :],
                    rhs=x8[:, j, :],
                    start=(j % TPW == 0),
                    stop=(j % TPW == TPW - 1),
                    skip_group_check=True,
                )

        def emit_centers():
            nc.scalar.activation(out=sq_c, in_=fc_sb, func=AF.Square)
            nc.vector.tensor_reduce(
                out=q_c, in_=sq_c, axis=mybir.AxisListType.X, op=OP.add
            )
            nc.scalar.activation(out=rt_c, in_=q_c, func=AF.Sqrt)
            nc.vector.reciprocal(out=inv_c, in_=rt_c)
            nc.gpsimd.tensor_tensor(
                out=cn_bf,
                in0=fc_sb,
                in1=inv_c.broadcast_to([CPW, W, D]),
                op=OP.mult,
            )

        def emit_s_and_T():
            for t in range(W):
                nc.tensor.matmul(
                    out=s_ps,
                    lhsT=ones_bf,
                    rhs=cn_bf[:, t, :],
                    start=(t == 0),
                    stop=(t == W - 1),
                    skip_group_check=True,
                )
            nc.vector.scalar_tensor_tensor(
                out=T_sb,
                in0=cn_bf,
                scalar=float(-C),
                op0=OP.mult,
                in1=s_ps[0:CPW, :].unsqueeze(1).broadcast_to([CPW, W, D]),
                op1=OP.add,
            )

        # ---------------- schedule ----------------
        emit_centers()
        emit_group(0)
        emit_s_and_T()
        emit_group(1)
        emit_group(2)
        # windows 0..5 are finished after group 2 (tiles 0..29 cover w0-5)
        nc.vector.scalar_tensor_tensor(
            out=scratch[:, 0:6, :],
            in0=g_ps[:, 0:6, :],
            scalar=1.0,
            op0=OP.mult,
            in1=T_sb[:, 0:6, :],
            op1=OP.mult,
            accum_out=res_sb[0:CPW, 0:1],
        )
        emit_group(3)
        nc.vector.scalar_tensor_tensor(
            out=scratch[:, 6:8, :],
            in0=g_ps[:, 6:8, :],
            scalar=1.0,
            op0=OP.mult,
            in1=T_sb[:, 6:8, :],
            op1=OP.mult,
            accum_out=res_sb[0:CPW, 1:2],
        )
        nc.sync.dma_start(out=out_res[:, :], in_=res_sb)

    return nc


def _get_nc() -> bass.Bass:
    global _CACHED_NC
    if _CACHED_NC is None:
        _CACHED_NC = build_nc()
        _CACHED_NC.finalize()
    return _CACHED_NC


def make_in_maps(hidden, feature_center, y):
    import ml_dtypes

    hidden = np.ascontiguousarray(np.asarray(hidden), dtype=np.float32)
    fc = np.ascontiguousarray(np.asarray(feature_center), dtype=np.float32)
    y64 = np.asarray(y).astype(np.int64)

    counts = np.bincount(y64, minlength=C)
    order = np.argsort(-counts, kind="stable")
    # Greedy: heaviest class first into the lightest window with room.
    # New label n = 8*pos + w  (so fc' loads contiguously per partition).
    wsum = np.zeros(W, dtype=np.int64)
    wlen = np.zeros(W, dtype=np.int64)
    relabel = np.empty(C, dtype=np.int64)
    for c in order:
        cands = np.nonzero(wlen < CPW)[0]
        w = cands[np.argmin(wsum[cands])]
        relabel[c] = 8 * wlen[w] + w
        wlen[w] += 1
        wsum[w] += counts[c]

    fc_prime = np.empty_like(fc)
    fc_prime[relabel] = fc
    ynew = relabel[y64]
    yw = ynew % W
    ylid = ynew // W

    fp8 = ml_dtypes.float8_e4m3
    hid_pads = []
    ohs = []
    for k in range(NCORES):
        hid_pads.append(np.tile(hidden[0], (BS_PAD, 1)))
        ohs.append(np.zeros((128, NT, CPW), dtype=np.float32))

    for w in range(W):
        idxs = np.nonzero(yw == w)[0]
        for k in range(NCORES):
            sub = idxs[k::NCORES]
            n = len(sub)
            assert n <= 128 * TPW, f"window {w} core {k} overflow: {n}"
            i = np.arange(n)
            p = i // TPW
            j = w * TPW + (i % TPW)
            hid_pads[k][NT * p + j] = hidden[sub]
            ohs[k][p, j, ylid[sub]] = 1.0

    in_maps = []
    for k in range(NCORES):
        in_maps.append(
            {
                "hidden": np.ascontiguousarray(hid_pads[k]),
                "fc": fc_prime,
                "oh": ohs[k].astype(fp8),
            }
        )
    return in_maps


def finish(results) -> np.ndarray:
    tot = 0.0
    for r in results:
        tot += np.asarray(r["res"], dtype=np.float64).sum()
    return np.float32(1.0 + tot / (B * (C - 1)))


def kernel(hidden, feature_center, y) -> np.ndarray:
    from concourse.bass_utils import run_bass_kernel_spmd

    in_maps = make_in_maps(hidden, feature_center, y)
    nc = _get_nc()
    res = run_bass_kernel_spmd(nc, in_maps, core_ids=list(range(NCORES)))
    return finish(res.results)


# revision 17
# speedup vs baseline: 1.0387x; 1.0387x over previous
"""ContrastiveCenterLoss Trainium2 Bass kernel — gather-free formulation.

Math (exact identities; see git history for derivation):
    cn_c = fc_c/||fc_c||;  s = sum_c cn_c;  X_b = hidden_b/||hidden_b||
    G_c  = sum_{b: y_b=c} X_b
    loss = 1 + sum_c G_c . (s - C*cn_c) / (B*(C-1))

The host relabels classes as n = 8*pos + w (window w, slot pos) so the
fc load is contiguous per partition, and bins each core's samples so
tile j only holds classes of window j//5. G accumulates on the PE via
fp8 one-hot matmuls (one-hot precomputed host-side — pure index
preprocessing). Pad slots duplicate a real row with a zero one-hot row.

DMA: h in 3 big chunks on sync (big chunks ~350GB/s vs 70GB/s for
small), fc contiguous on scalar, one-hot fp8 on the gpsimd SWDGE queue.
"""

import sys

sys.path.insert(0, "/opt/trn_rl_repo")

from contextlib import ExitStack

import numpy as np

import concourse.bass as bass
import concourse.tile as tile
from concourse import bacc, mybir

B, C, D = 32768, 1000, 128
NCORES = 8
W = 8                     # class windows
CPW = C // W              # 125 classes per window
TPW = 5                   # tiles per window
NT = W * TPW              # 40 tiles of 128 slots per core
BS_PAD = NT * 128         # 5120 padded slots per core
GROUPS = [(0, 12), (12, 24), (24, 36), (36, 40)]
F32 = mybir.dt.float32
BF16 = mybir.dt.bfloat16
FP8 = mybir.dt.float8e4

_CACHED_NC = None


def build_nc() -> bass.Bass:
    AF = mybir.ActivationFunctionType
    OP = mybir.AluOpType

    nc = bacc.Bacc()
    hidden = nc.dram_tensor("hidden", [BS_PAD, D], F32, kind="ExternalInput")
    fc = nc.dram_tensor("fc", [C, D], F32, kind="ExternalInput")
    oh_t = nc.dram_tensor("oh", [128, NT, CPW], FP8, kind="ExternalInput")
    out_res = nc.dram_tensor("res", [128, 2], F32, kind="ExternalOutput")

    with tile.TileContext(nc) as tc, ExitStack() as ctx:
        singles = ctx.enter_context(tc.tile_pool(name="singles", bufs=1))
        psum = ctx.enter_context(tc.tile_pool(name="psum", bufs=1, space="PSUM"))

        # ---------------- DMA in ----------------
        # fc FIRST on sync (its arrival gates the centers->T->finals chain);
        # h group-chunks split across both HWDGE queues; one-hot on SWDGE.
        # Balance bytes across the three DMA paths at their measured
        # in-kernel rates; fc first on the fastest queue (it gates the
        # centers->T->finals chain).
        fc_sb = singles.tile([CPW, W, D], F32)
        h_all = singles.tile([128, NT, D], F32)
        oh = singles.tile([128, NT, CPW], FP8)
        h_src = hidden[:, :].rearrange("(p i) d -> p i d", p=128)

        nc.scalar.dma_start(
            out=fc_sb, in_=fc[:, :].rearrange("(p t) d -> p t d", t=W)
        )
        nc.sync.dma_start(out=h_all[:, 0:12, :], in_=h_src[:, 0:12, :])    # g0
        nc.gpsimd.dma_start(out=oh[:, 0:24, :], in_=oh_t[:, 0:24, :])
        nc.scalar.dma_start(out=h_all[:, 12:24, :], in_=h_src[:, 12:24, :])  # g1
        nc.gpsimd.dma_start(out=oh[:, 24:NT, :], in_=oh_t[:, 24:NT, :])
        nc.sync.dma_start(out=h_all[:, 36:NT, :], in_=h_src[:, 36:NT, :])  # g3
        nc.gpsimd.dma_start(out=h_all[:, 24:36, :], in_=h_src[:, 24:36, :])  # g2

        # Preload both ACT tables during the DMA window.
        warm = singles.tile([128, 1], F32)
        nc.scalar.activation(out=warm, in_=warm, func=AF.Sqrt, scale=0.0, bias=1.0)
        nc.scalar.activation(out=warm, in_=warm, func=AF.Square)

        # ---------------- tiles ----------------
        sq = singles.tile([128, NT, D], BF16)
        q_all = singles.tile([128, NT], BF16)
        rt_all = singles.tile([128, NT], BF16)
        invh = singles.tile([128, NT], BF16)
        x8 = singles.tile([128, NT, D], FP8)
        g_ps = psum.tile([CPW, W, D], F32)
        s_ps = psum.tile([128, D], F32)
        res_sb = singles.tile([128, 2], F32)
        nc.vector.memset(res_sb, 0.0)
        scratch = singles.tile([CPW, W, D], F32)
        T_sb = singles.tile([CPW, W, D], F32)

        sq_c = singles.tile([CPW, W, D], BF16)
        q_c = singles.tile([CPW, W], F32)
        rt_c = singles.tile([CPW, W], F32)
        inv_c = singles.tile([CPW, W], F32)
        cn_bf = singles.tile([CPW, W, D], BF16)
        ones_bf = singles.tile([CPW, 128], BF16)
        nc.vector.memset(ones_bf, 1.0)

        def emit_group(gi):
            j0, j1 = GROUPS[gi]
            n = j1 - j0
            nc.scalar.activation(
                out=sq[:, j0:j1, :], in_=h_all[:, j0:j1, :], func=AF.Square
            )
            with nc.allow_low_precision("q in bf16: 0.4% on norms is fine"):
                nc.vector.tensor_reduce(
                    out=q_all[:, j0:j1],
                    in_=sq[:, j0:j1, :],
                    axis=mybir.AxisListType.X,
                    op=OP.add,
                )
            nc.scalar.activation(
                out=rt_all[:, j0:j1], in_=q_all[:, j0:j1], func=AF.Sqrt
            )
            with nc.allow_low_precision("invh in bf16: fine for cosine tol"):
                nc.vector.reciprocal(out=invh[:, j0:j1], in_=rt_all[:, j0:j1])
            # X = h/||h|| in fp8 (PE operand); g0/g1 on gpsimd, g2/g3 DVE
            eng = nc.gpsimd if gi < 2 else nc.vector
            eng.tensor_tensor(
                out=x8[:, j0:j1, :],
                in0=h_all[:, j0:j1, :],
                in1=invh[:, j0:j1].broadcast_to([128, n, D]),
                op=OP.mult,
            )
            for j in range(j0, j1):
                w = j // TPW
                nc.tensor.matmul(
                    out=g_ps[:, w, :],
                    lhsT=oh[:, j, :],
                    rhs=x8[:, j, :],
                    start=(j % TPW == 0),
                    stop=(j % TPW == TPW - 1),
                    skip_group_check=True,
                )

        def emit_centers():
            nc.scalar.activation(out=sq_c, in_=fc_sb, func=AF.Square)
            nc.vector.tensor_reduce(
                out=q_c, in_=sq_c, axis=mybir.AxisListType.X, op=OP.add
            )
            nc.scalar.activation(out=rt_c, in_=q_c, func=AF.Sqrt)
            nc.vector.reciprocal(out=inv_c, in_=rt_c)
            nc.gpsimd.tensor_tensor(
                out=cn_bf,
                in0=fc_sb,
                in1=inv_c.broadcast_to([CPW, W, D]),
                op=OP.mult,
            )

        def emit_s_and_T():
            for t in range(W):
                nc.tensor.matmul(
                    out=s_ps,
                    lhsT=ones_bf,
                    rhs=cn_bf[:, t, :],
                    start=(t == 0),
                    stop=(t == W - 1),
                    skip_group_check=True,
                )
            nc.vector.scalar_tensor_tensor(
                out=T_sb,
                in0=cn_bf,
                scalar=float(-C),
                op0=OP.mult,
                in1=s_ps[0:CPW, :].unsqueeze(1).broadcast_to([CPW, W, D]),
                op1=OP.add,
            )

        # ---------------- schedule ----------------
        emit_centers()
        emit_group(0)
        emit_s_and_T()
        emit_group(1)
        emit_group(2)
        # windows 0..5 are finished after group 2 (tiles 0..29 cover w0-5)
        nc.vector.scalar_tensor_tensor(
            out=scratch[:, 0:6, :],
            in0=g_ps[:, 0:6, :],
            scalar=1.0,
            op0=OP.mult,
            in1=T_sb[:, 0:6, :],
            op1=OP.mult,
            accum_out=res_sb[0:CPW, 0:1],
        )
        emit_group(3)
        nc.vector.scalar_tensor_tensor(
            out=scratch[:, 6:8, :],
            in0=g_ps[:, 6:8, :],
            scalar=1.0,
            op0=OP.mult,
            in1=T_sb[:, 6:8, :],
            op1=OP.mult,
            accum_out=res_sb[0:CPW, 1:2],
        )
        nc.sync.dma_start(out=out_res[:, :], in_=res_sb)

    return nc


def _get_nc() -> bass.Bass:
    global _CACHED_NC
    if _CACHED_NC is None:
        _CACHED_NC = build_nc()
        _CACHED_NC.finalize()
    return _CACHED_NC


def make_in_maps(hidden, feature_center, y):
    import ml_dtypes

    hidden = np.ascontiguousarray(np.asarray(hidden), dtype=np.float32)
    fc = np.ascontiguousarray(np.asarray(feature_center), dtype=np.float32)
    y64 = np.asarray(y).astype(np.int64)

    counts = np.bincount(y64, minlength=C)
    order = np.argsort(-counts, kind="stable")
    # Greedy: heaviest class first into the lightest window with room.
    # New label n = 8*pos + w  (so fc' loads contiguously per partition).
    wsum = np.zeros(W, dtype=np.int64)
    wlen = np.zeros(W, dtype=np.int64)
    relabel = np.empty(C, dtype=np.int64)
    for c in order:
        cands = np.nonzero(wlen < CPW)[0]
        w = cands[np.argmin(wsum[cands])]
        relabel[c] = 8 * wlen[w] + w
        wlen[w] += 1
        wsum[w] += counts[c]

    fc_prime = np.empty_like(fc)
    fc_prime[relabel] = fc
    ynew = relabel[y64]
    yw = ynew % W
    ylid = ynew // W

    fp8 = ml_dtypes.float8_e4m3
    hid_pads = []
    ohs = []
    for k in range(NCORES):
        hid_pads.append(np.tile(hidden[0], (BS_PAD, 1)))
        ohs.append(np.zeros((128, NT, CPW), dtype=np.float32))

    for w in range(W):
        idxs = np.nonzero(yw == w)[0]
        for k in range(NCORES):
            sub = idxs[k::NCORES]
            n = len(sub)
            assert n <= 128 * TPW, f"window {w} core {k} overflow: {n}"
            i = np.arange(n)
            p = i // TPW
            j = w * TPW + (i % TPW)
            hid_pads[k][NT * p + j] = hidden[sub]
            ohs[k][p, j, ylid[sub]] = 1.0

    in_maps = []
    for k in range(NCORES):
        in_maps.append(
            {
                "hidden": np.ascontiguousarray(hid_pads[k]),
                "fc": fc_prime,
                "oh": ohs[k].astype(fp8),
            }
        )
    return in_maps


def finish(results) -> np.ndarray:
    tot = 0.0
    for r in results:
        tot += np.asarray(r["res"], dtype=np.float64).sum()
    return np.float32(1.0 + tot / (B * (C - 1)))


def kernel(hidden, feature_center, y) -> np.ndarray:
    from concourse.bass_utils import run_bass_kernel_spmd

    in_maps = make_in_maps(hidden, feature_center, y)
    nc = _get_nc()
    res = run_bass_kernel_spmd(nc, in_maps, core_ids=list(range(NCORES)))
    return finish(res.results)


# revision 18
# speedup vs baseline: 1.0633x; 1.0237x over previous
"""ContrastiveCenterLoss Trainium2 Bass kernel — gather-free formulation.

Math (exact identities; see git history for derivation):
    cn_c = fc_c/||fc_c||;  s = sum_c cn_c;  X_b = hidden_b/||hidden_b||
    G_c  = sum_{b: y_b=c} X_b
    loss = 1 + sum_c G_c . (s - C*cn_c) / (B*(C-1))

The host relabels classes as n = 8*pos + w (window w, slot pos) so the
fc load is contiguous per partition, and bins each core's samples so
tile j only holds classes of window j//5. G accumulates on the PE via
fp8 one-hot matmuls (one-hot precomputed host-side — pure index
preprocessing). Pad slots duplicate a real row with a zero one-hot row.

DMA: h in 3 big chunks on sync (big chunks ~350GB/s vs 70GB/s for
small), fc contiguous on scalar, one-hot fp8 on the gpsimd SWDGE queue.
"""

import sys

sys.path.insert(0, "/opt/trn_rl_repo")

from contextlib import ExitStack

import numpy as np

import concourse.bass as bass
import concourse.tile as tile
from concourse import bacc, mybir

B, C, D = 32768, 1000, 128
NCORES = 8
W = 8                     # class windows
CPW = C // W              # 125 classes per window
TPW = 5                   # tiles per window
NT = W * TPW              # 40 tiles of 128 slots per core
BS_PAD = NT * 128         # 5120 padded slots per core
GROUPS = [(0, 12), (12, 24), (24, 36), (36, 40)]
F32 = mybir.dt.float32
BF16 = mybir.dt.bfloat16
FP8 = mybir.dt.float8e4

_CACHED_NC = None


def build_nc() -> bass.Bass:
    AF = mybir.ActivationFunctionType
    OP = mybir.AluOpType

    nc = bacc.Bacc()
    hidden = nc.dram_tensor("hidden", [BS_PAD, D], F32, kind="ExternalInput")
    fc = nc.dram_tensor("fc", [C, D], F32, kind="ExternalInput")
    oh_t = nc.dram_tensor("oh", [128, NT, CPW], FP8, kind="ExternalInput")
    out_res = nc.dram_tensor("res", [128, 2], F32, kind="ExternalOutput")

    with tile.TileContext(nc) as tc, ExitStack() as ctx:
        singles = ctx.enter_context(tc.tile_pool(name="singles", bufs=1))
        psum = ctx.enter_context(tc.tile_pool(name="psum", bufs=1, space="PSUM"))

        # ---------------- DMA in ----------------
        # fc FIRST on sync (its arrival gates the centers->T->finals chain);
        # h group-chunks split across both HWDGE queues; one-hot on SWDGE.
        # Balance bytes across the three DMA paths at their measured
        # in-kernel rates; fc first on the fastest queue (it gates the
        # centers->T->finals chain).
        fc_sb = singles.tile([CPW, W, D], F32)
        h_all = singles.tile([128, NT, D], F32)
        oh = singles.tile([128, NT, CPW], FP8)
        h_src = hidden[:, :].rearrange("(p i) d -> p i d", p=128)

        # sync HWDGE is pathologically slow in-kernel (~32GB/s observed);
        # keep ALL input loads on scalar HWDGE (~103) + gpsimd SWDGE (~135).
        nc.scalar.dma_start(
            out=fc_sb, in_=fc[:, :].rearrange("(p t) d -> p t d", t=W)
        )
        nc.gpsimd.dma_start(out=oh[:, 0:24, :], in_=oh_t[:, 0:24, :])
        nc.gpsimd.dma_start(out=h_all[:, 0:12, :], in_=h_src[:, 0:12, :])    # g0
        nc.scalar.dma_start(out=h_all[:, 12:24, :], in_=h_src[:, 12:24, :])  # g1
        nc.gpsimd.dma_start(out=oh[:, 24:NT, :], in_=oh_t[:, 24:NT, :])
        nc.scalar.dma_start(out=h_all[:, 36:NT, :], in_=h_src[:, 36:NT, :])  # g3
        nc.gpsimd.dma_start(out=h_all[:, 24:36, :], in_=h_src[:, 24:36, :])  # g2

        # Preload both ACT tables during the DMA window.
        warm = singles.tile([128, 1], F32)
        nc.scalar.activation(out=warm, in_=warm, func=AF.Sqrt, scale=0.0, bias=1.0)
        nc.scalar.activation(out=warm, in_=warm, func=AF.Square)

        # ---------------- tiles ----------------
        sq = singles.tile([128, NT, D], BF16)
        q_all = singles.tile([128, NT], BF16)
        rt_all = singles.tile([128, NT], BF16)
        invh = singles.tile([128, NT], BF16)
        x8 = singles.tile([128, NT, D], FP8)
        g_ps = psum.tile([CPW, W, D], F32)
        s_ps = psum.tile([128, D], F32)
        res_sb = singles.tile([128, 2], F32)
        nc.vector.memset(res_sb, 0.0)
        scratch = singles.tile([CPW, W, D], F32)
        T_sb = singles.tile([CPW, W, D], F32)

        sq_c = singles.tile([CPW, W, D], BF16)
        q_c = singles.tile([CPW, W], F32)
        rt_c = singles.tile([CPW, W], F32)
        inv_c = singles.tile([CPW, W], F32)
        cn_bf = singles.tile([CPW, W, D], BF16)
        ones_bf = singles.tile([CPW, 128], BF16)
        nc.vector.memset(ones_bf, 1.0)

        def emit_group(gi):
            j0, j1 = GROUPS[gi]
            n = j1 - j0
            nc.scalar.activation(
                out=sq[:, j0:j1, :], in_=h_all[:, j0:j1, :], func=AF.Square
            )
            with nc.allow_low_precision("q in bf16: 0.4% on norms is fine"):
                nc.vector.tensor_reduce(
                    out=q_all[:, j0:j1],
                    in_=sq[:, j0:j1, :],
                    axis=mybir.AxisListType.X,
                    op=OP.add,
                )
            nc.scalar.activation(
                out=rt_all[:, j0:j1], in_=q_all[:, j0:j1], func=AF.Sqrt
            )
            with nc.allow_low_precision("invh in bf16: fine for cosine tol"):
                nc.vector.reciprocal(out=invh[:, j0:j1], in_=rt_all[:, j0:j1])
            # X = h/||h|| in fp8 (PE operand); g0/g1 on gpsimd, g2/g3 DVE
            eng = nc.gpsimd if gi < 2 else nc.vector
            eng.tensor_tensor(
                out=x8[:, j0:j1, :],
                in0=h_all[:, j0:j1, :],
                in1=invh[:, j0:j1].broadcast_to([128, n, D]),
                op=OP.mult,
            )
            for j in range(j0, j1):
                w = j // TPW
                nc.tensor.matmul(
                    out=g_ps[:, w, :],
                    lhsT=oh[:, j, :],
                    rhs=x8[:, j, :],
                    start=(j % TPW == 0),
                    stop=(j % TPW == TPW - 1),
                    skip_group_check=True,
                )

        def emit_centers():
            nc.scalar.activation(out=sq_c, in_=fc_sb, func=AF.Square)
            nc.vector.tensor_reduce(
                out=q_c, in_=sq_c, axis=mybir.AxisListType.X, op=OP.add
            )
            nc.scalar.activation(out=rt_c, in_=q_c, func=AF.Sqrt)
            nc.vector.reciprocal(out=inv_c, in_=rt_c)
            nc.gpsimd.tensor_tensor(
                out=cn_bf,
                in0=fc_sb,
                in1=inv_c.broadcast_to([CPW, W, D]),
                op=OP.mult,
            )

        def emit_s_and_T():
            for t in range(W):
                nc.tensor.matmul(
                    out=s_ps,
                    lhsT=ones_bf,
                    rhs=cn_bf[:, t, :],
                    start=(t == 0),
                    stop=(t == W - 1),
                    skip_group_check=True,
                )
            nc.vector.scalar_tensor_tensor(
                out=T_sb,
                in0=cn_bf,
                scalar=float(-C),
                op0=OP.mult,
                in1=s_ps[0:CPW, :].unsqueeze(1).broadcast_to([CPW, W, D]),
                op1=OP.add,
            )

        # ---------------- schedule ----------------
        emit_centers()
        emit_group(0)
        emit_s_and_T()
        emit_group(1)
        emit_group(2)
        # windows 0..5 are finished after group 2 (tiles 0..29 cover w0-5)
        nc.vector.scalar_tensor_tensor(
            out=scratch[:, 0:6, :],
            in0=g_ps[:, 0:6, :],
            scalar=1.0,
            op0=OP.mult,
            in1=T_sb[:, 0:6, :],
            op1=OP.mult,
            accum_out=res_sb[0:CPW, 0:1],
        )
        emit_group(3)
        nc.vector.scalar_tensor_tensor(
            out=scratch[:, 6:8, :],
            in0=g_ps[:, 6:8, :],
            scalar=1.0,
            op0=OP.mult,
            in1=T_sb[:, 6:8, :],
            op1=OP.mult,
            accum_out=res_sb[0:CPW, 1:2],
        )
        nc.sync.dma_start(out=out_res[:, :], in_=res_sb)

    return nc


def _get_nc() -> bass.Bass:
    global _CACHED_NC
    if _CACHED_NC is None:
        _CACHED_NC = build_nc()
        _CACHED_NC.finalize()
    return _CACHED_NC


def make_in_maps(hidden, feature_center, y):
    import ml_dtypes

    hidden = np.ascontiguousarray(np.asarray(hidden), dtype=np.float32)
    fc = np.ascontiguousarray(np.asarray(feature_center), dtype=np.float32)
    y64 = np.asarray(y).astype(np.int64)

    counts = np.bincount(y64, minlength=C)
    order = np.argsort(-counts, kind="stable")
    # Greedy: heaviest class first into the lightest window with room.
    # New label n = 8*pos + w  (so fc' loads contiguously per partition).
    wsum = np.zeros(W, dtype=np.int64)
    wlen = np.zeros(W, dtype=np.int64)
    relabel = np.empty(C, dtype=np.int64)
    for c in order:
        cands = np.nonzero(wlen < CPW)[0]
        w = cands[np.argmin(wsum[cands])]
        relabel[c] = 8 * wlen[w] + w
        wlen[w] += 1
        wsum[w] += counts[c]

    fc_prime = np.empty_like(fc)
    fc_prime[relabel] = fc
    ynew = relabel[y64]
    yw = ynew % W
    ylid = ynew // W

    fp8 = ml_dtypes.float8_e4m3
    hid_pads = []
    ohs = []
    for k in range(NCORES):
        hid_pads.append(np.tile(hidden[0], (BS_PAD, 1)))
        ohs.append(np.zeros((128, NT, CPW), dtype=np.float32))

    for w in range(W):
        idxs = np.nonzero(yw == w)[0]
        for k in range(NCORES):
            sub = idxs[k::NCORES]
            n = len(sub)
            assert n <= 128 * TPW, f"window {w} core {k} overflow: {n}"
            i = np.arange(n)
            p = i // TPW
            j = w * TPW + (i % TPW)
            hid_pads[k][NT * p + j] = hidden[sub]
            ohs[k][p, j, ylid[sub]] = 1.0

    in_maps = []
    for k in range(NCORES):
        in_maps.append(
            {
                "hidden": np.ascontiguousarray(hid_pads[k]),
                "fc": fc_prime,
                "oh": ohs[k].astype(fp8),
            }
        )
    return in_maps


def finish(results) -> np.ndarray:
    tot = 0.0
    for r in results:
        tot += np.asarray(r["res"], dtype=np.float64).sum()
    return np.float32(1.0 + tot / (B * (C - 1)))


def kernel(hidden, feature_center, y) -> np.ndarray:
    from concourse.bass_utils import run_bass_kernel_spmd

    in_maps = make_in_maps(hidden, feature_center, y)
    nc = _get_nc()
    res = run_bass_kernel_spmd(nc, in_maps, core_ids=list(range(NCORES)))
    return finish(res.results)


# revision 19
# speedup vs baseline: 1.2290x; 1.1558x over previous
"""ContrastiveCenterLoss Trainium2 Bass kernel — gather-free formulation.

Math (exact identities; see git history for derivation):
    cn_c = fc_c/||fc_c||;  s = sum_c cn_c;  X_b = hidden_b/||hidden_b||
    G_c  = sum_{b: y_b=c} X_b
    loss = 1 + sum_c G_c . (s - C*cn_c) / (B*(C-1))

The host relabels classes as n = 8*pos + w (window w, slot pos) so the
fc load is contiguous per partition, and bins each core's samples so
tile j only holds classes of window j//5. G accumulates on the PE via
fp8 one-hot matmuls (one-hot precomputed host-side — pure index
preprocessing). Pad slots duplicate a real row with a zero one-hot row.

DMA: h in 3 big chunks on sync (big chunks ~350GB/s vs 70GB/s for
small), fc contiguous on scalar, one-hot fp8 on the gpsimd SWDGE queue.
"""

import sys

sys.path.insert(0, "/opt/trn_rl_repo")

from contextlib import ExitStack

import numpy as np

import concourse.bass as bass
import concourse.tile as tile
from concourse import bacc, mybir

B, C, D = 32768, 1000, 128
NCORES = 8
W = 8                     # class windows
CPW = C // W              # 125 classes per window
TPW = 5                   # tiles per window
NT = W * TPW              # 40 tiles of 128 slots per core
BS_PAD = NT * 128         # 5120 padded slots per core
GROUPS = [(0, 12), (12, 24), (24, 36), (36, 40)]
F32 = mybir.dt.float32
BF16 = mybir.dt.bfloat16
FP8 = mybir.dt.float8e4

_CACHED_NC = None


def build_nc() -> bass.Bass:
    AF = mybir.ActivationFunctionType
    OP = mybir.AluOpType

    nc = bacc.Bacc()
    hidden = nc.dram_tensor("hidden", [BS_PAD, D], BF16, kind="ExternalInput")
    fc = nc.dram_tensor("fc", [C, D], BF16, kind="ExternalInput")
    oh_t = nc.dram_tensor("oh", [128, NT, CPW], FP8, kind="ExternalInput")
    out_res = nc.dram_tensor("res", [128, 2], F32, kind="ExternalOutput")

    with tile.TileContext(nc) as tc, ExitStack() as ctx:
        singles = ctx.enter_context(tc.tile_pool(name="singles", bufs=1))
        psum = ctx.enter_context(tc.tile_pool(name="psum", bufs=1, space="PSUM"))

        # ---------------- DMA in ----------------
        # fc FIRST on sync (its arrival gates the centers->T->finals chain);
        # h group-chunks split across both HWDGE queues; one-hot on SWDGE.
        # Balance bytes across the three DMA paths at their measured
        # in-kernel rates; fc first on the fastest queue (it gates the
        # centers->T->finals chain).
        fc_sb = singles.tile([CPW, W, D], BF16)
        h_all = singles.tile([128, NT, D], BF16)
        oh = singles.tile([128, NT, CPW], FP8)
        h_src = hidden[:, :].rearrange("(p i) d -> p i d", p=128)

        # sync HWDGE is pathologically slow in-kernel (~32GB/s observed);
        # keep ALL input loads on scalar HWDGE (~103) + gpsimd SWDGE (~135).
        nc.scalar.dma_start(
            out=fc_sb, in_=fc[:, :].rearrange("(p t) d -> p t d", t=W)
        )
        nc.gpsimd.dma_start(out=oh[:, 0:24, :], in_=oh_t[:, 0:24, :])
        nc.gpsimd.dma_start(out=h_all[:, 0:12, :], in_=h_src[:, 0:12, :])    # g0
        nc.scalar.dma_start(out=h_all[:, 12:24, :], in_=h_src[:, 12:24, :])  # g1
        nc.gpsimd.dma_start(out=oh[:, 24:NT, :], in_=oh_t[:, 24:NT, :])
        nc.scalar.dma_start(out=h_all[:, 36:NT, :], in_=h_src[:, 36:NT, :])  # g3
        nc.gpsimd.dma_start(out=h_all[:, 24:36, :], in_=h_src[:, 24:36, :])  # g2

        # Preload both ACT tables during the DMA window.
        warm = singles.tile([128, 1], F32)
        nc.scalar.activation(out=warm, in_=warm, func=AF.Sqrt, scale=0.0, bias=1.0)
        nc.scalar.activation(out=warm, in_=warm, func=AF.Square)

        # ---------------- tiles ----------------
        sq = singles.tile([128, NT, D], BF16)
        q_all = singles.tile([128, NT], BF16)
        rt_all = singles.tile([128, NT], BF16)
        invh = singles.tile([128, NT], BF16)
        x8 = singles.tile([128, NT, D], FP8)
        g_ps = psum.tile([CPW, W, D], F32)
        s_ps = psum.tile([128, D], F32)
        res_sb = singles.tile([128, 2], F32)
        nc.vector.memset(res_sb, 0.0)
        scratch = singles.tile([CPW, W, D], F32)
        T_sb = singles.tile([CPW, W, D], F32)

        sq_c = singles.tile([CPW, W, D], BF16)
        q_c = singles.tile([CPW, W], F32)
        rt_c = singles.tile([CPW, W], F32)
        inv_c = singles.tile([CPW, W], F32)
        cn_bf = singles.tile([CPW, W, D], BF16)
        ones_bf = singles.tile([CPW, 128], BF16)
        nc.vector.memset(ones_bf, 1.0)

        def emit_group(gi):
            j0, j1 = GROUPS[gi]
            n = j1 - j0
            nc.scalar.activation(
                out=sq[:, j0:j1, :], in_=h_all[:, j0:j1, :], func=AF.Square
            )
            with nc.allow_low_precision("q in bf16: 0.4% on norms is fine"):
                nc.vector.tensor_reduce(
                    out=q_all[:, j0:j1],
                    in_=sq[:, j0:j1, :],
                    axis=mybir.AxisListType.X,
                    op=OP.add,
                )
            nc.scalar.activation(
                out=rt_all[:, j0:j1], in_=q_all[:, j0:j1], func=AF.Sqrt
            )
            with nc.allow_low_precision("invh in bf16: fine for cosine tol"):
                nc.vector.reciprocal(out=invh[:, j0:j1], in_=rt_all[:, j0:j1])
            # X = h/||h|| in fp8 (PE operand); g0/g1 on gpsimd, g2/g3 DVE
            eng = nc.gpsimd if gi < 2 else nc.vector
            eng.tensor_tensor(
                out=x8[:, j0:j1, :],
                in0=h_all[:, j0:j1, :],
                in1=invh[:, j0:j1].broadcast_to([128, n, D]),
                op=OP.mult,
            )
            for j in range(j0, j1):
                w = j // TPW
                nc.tensor.matmul(
                    out=g_ps[:, w, :],
                    lhsT=oh[:, j, :],
                    rhs=x8[:, j, :],
                    start=(j % TPW == 0),
                    stop=(j % TPW == TPW - 1),
                    skip_group_check=True,
                )

        def emit_centers():
            nc.scalar.activation(out=sq_c, in_=fc_sb, func=AF.Square)
            nc.vector.tensor_reduce(
                out=q_c, in_=sq_c, axis=mybir.AxisListType.X, op=OP.add
            )
            nc.scalar.activation(out=rt_c, in_=q_c, func=AF.Sqrt)
            nc.vector.reciprocal(out=inv_c, in_=rt_c)
            nc.vector.tensor_tensor(
                out=cn_bf,
                in0=fc_sb,
                in1=inv_c.broadcast_to([CPW, W, D]),
                op=OP.mult,
            )

        def emit_s_and_T():
            for t in range(W):
                nc.tensor.matmul(
                    out=s_ps,
                    lhsT=ones_bf,
                    rhs=cn_bf[:, t, :],
                    start=(t == 0),
                    stop=(t == W - 1),
                    skip_group_check=True,
                )
            nc.vector.scalar_tensor_tensor(
                out=T_sb,
                in0=cn_bf,
                scalar=float(-C),
                op0=OP.mult,
                in1=s_ps[0:CPW, :].unsqueeze(1).broadcast_to([CPW, W, D]),
                op1=OP.add,
            )

        # ---------------- schedule ----------------
        emit_centers()
        emit_group(0)
        emit_s_and_T()
        emit_group(1)
        emit_group(2)
        # windows 0..5 are finished after group 2 (tiles 0..29 cover w0-5)
        nc.vector.scalar_tensor_tensor(
            out=scratch[:, 0:6, :],
            in0=g_ps[:, 0:6, :],
            scalar=1.0,
            op0=OP.mult,
            in1=T_sb[:, 0:6, :],
            op1=OP.mult,
            accum_out=res_sb[0:CPW, 0:1],
        )
        emit_group(3)
        nc.vector.scalar_tensor_tensor(
            out=scratch[:, 6:8, :],
            in0=g_ps[:, 6:8, :],
            scalar=1.0,
            op0=OP.mult,
            in1=T_sb[:, 6:8, :],
            op1=OP.mult,
            accum_out=res_sb[0:CPW, 1:2],
        )
        nc.sync.dma_start(out=out_res[:, :], in_=res_sb)

    return nc


def _get_nc() -> bass.Bass:
    global _CACHED_NC
    if _CACHED_NC is None:
        _CACHED_NC = build_nc()
        _CACHED_NC.finalize()
    return _CACHED_NC


def make_in_maps(hidden, feature_center, y):
    import ml_dtypes

    hidden = np.ascontiguousarray(np.asarray(hidden), dtype=np.float32)
    fc = np.ascontiguousarray(np.asarray(feature_center), dtype=np.float32)
    y64 = np.asarray(y).astype(np.int64)

    counts = np.bincount(y64, minlength=C)
    order = np.argsort(-counts, kind="stable")
    # Greedy: heaviest class first into the lightest window with room.
    # New label n = 8*pos + w  (so fc' loads contiguously per partition).
    wsum = np.zeros(W, dtype=np.int64)
    wlen = np.zeros(W, dtype=np.int64)
    relabel = np.empty(C, dtype=np.int64)
    for c in order:
        cands = np.nonzero(wlen < CPW)[0]
        w = cands[np.argmin(wsum[cands])]
        relabel[c] = 8 * wlen[w] + w
        wlen[w] += 1
        wsum[w] += counts[c]

    fc_prime = np.empty_like(fc)
    fc_prime[relabel] = fc
    ynew = relabel[y64]
    yw = ynew % W
    ylid = ynew // W

    fp8 = ml_dtypes.float8_e4m3
    hid_pads = []
    ohs = []
    for k in range(NCORES):
        hid_pads.append(np.tile(hidden[0], (BS_PAD, 1)))
        ohs.append(np.zeros((128, NT, CPW), dtype=np.float32))

    for w in range(W):
        idxs = np.nonzero(yw == w)[0]
        for k in range(NCORES):
            sub = idxs[k::NCORES]
            n = len(sub)
            assert n <= 128 * TPW, f"window {w} core {k} overflow: {n}"
            i = np.arange(n)
            p = i // TPW
            j = w * TPW + (i % TPW)
            hid_pads[k][NT * p + j] = hidden[sub]
            ohs[k][p, j, ylid[sub]] = 1.0

    in_maps = []
    for k in range(NCORES):
        in_maps.append(
            {
                "hidden": np.ascontiguousarray(hid_pads[k]).astype(ml_dtypes.bfloat16),
                "fc": fc_prime.astype(ml_dtypes.bfloat16),
                "oh": ohs[k].astype(fp8),
            }
        )
    return in_maps


def finish(results) -> np.ndarray:
    tot = 0.0
    for r in results:
        tot += np.asarray(r["res"], dtype=np.float64).sum()
    return np.float32(1.0 + tot / (B * (C - 1)))


def kernel(hidden, feature_center, y) -> np.ndarray:
    from concourse.bass_utils import run_bass_kernel_spmd

    in_maps = make_in_maps(hidden, feature_center, y)
    nc = _get_nc()
    res = run_bass_kernel_spmd(nc, in_maps, core_ids=list(range(NCORES)))
    return finish(res.results)
